# revision 11
# baseline (speedup 1.0000x reference)
"""CARAFE content-aware upsampling on 8 Trainium2 NeuronCores.

Strategy (data parallel): 8 cores = 4 batch images x 2 row-halves
(32 low-res rows each, +2-row halo). Per core, fused pipeline in SBUF:
  A) y_down = conv1x1(x, w_down)+b_down       (PE, K=256 in 2 chunks)
  B) enc -> exp(enc+b_enc)                    (PE conv3x3, 9 shifted matmuls)
  C) mask = softmax over 25 taps (4 groups)   (PE selector matmul -> DVE
     reciprocal+normalize), scattered to a banded DRAM image
  Z) zT = (w_out . x) transposed              (PE produces [col, ch])
  D) out = sum_i zt[h+i] @ B_band(h,i) + b_out (PE, K=68 banded matmuls,
     one MM per (h, tap, ch-half), N=256)
The final 1x1 conv (w_out) is folded BEFORE reassembly (z-trick).

v2 changes vs v1: K=68 single-band stage D (320 MMs vs 1280), PE warmup
spin to hold HAM at 2.4GHz, dense A->B->C||Z->D ordering, bf16 output,
consolidated weight-blob loads, 1 scatter DMA per kc, chunky reloads,
8-slot rotating band image zero-initialized once during the preamble.

Layouts:
  xs     [256, 36, 68]   zero-padded shard (rows h0-2..h1+2, cols -2..65)
  zt     [68, 36, 256]   col-on-partition transpose of z = w_out . x
  btX    [68, 8, 1280]   banded masks, 8 rotating h-slots:
                         btX[w+j, h%8, w*20 + i*4 + p] = mask[h,w,i,j,p]
  out    [256, 64, 128]  hi-res shard (bf16)
"""

import sys
import functools
import numpy as np
from contextlib import ExitStack

for _p in ("/opt/trn_rl_repo",):
    if _p not in sys.path:
        sys.path.insert(0, _p)

import concourse.bass as bass
import concourse.bacc as bacc
import concourse.mybir as mybir
import concourse.tile as tile
from concourse.bass_utils import run_bass_kernel_spmd

NCORES = 8
FP = mybir.dt.float32
BF = mybir.dt.bfloat16
AF = mybir.ActivationFunctionType
ALU = mybir.AluOpType


def _ap(base, offset_delta, dims):
    return bass.AP(tensor=base.tensor, offset=base.offset + offset_delta, ap=dims)


@functools.lru_cache(maxsize=1)
def _build():
    nc = bacc.Bacc("TRN2", target_bir_lowering=False, debug=False, num_devices=NCORES)

    xs_d = nc.declare_dram_parameter("xs", [256, 36, 68], BF, isOutput=False)
    wb_d = nc.declare_dram_parameter("wb", [128, 1772], BF, isOutput=False)
    fb_d = nc.declare_dram_parameter("fb", [128, 6], FP, isOutput=False)
    out_d = nc.declare_dram_parameter("out", [256, 64, 128], BF, isOutput=True)

    with tile.TileContext(nc) as tc:
        with ExitStack() as ctx:
            const = ctx.enter_context(tc.tile_pool(name="const", bufs=1))
            big = ctx.enter_context(tc.tile_pool(name="big", bufs=1))
            opool = ctx.enter_context(tc.tile_pool(name="opool", bufs=4))
            dpool = ctx.enter_context(tc.tile_pool(name="dpool", bufs=1, space="DRAM"))

            # ---- tiles ----
            warm = const.tile([128, 128], BF)
            zero_b = const.tile([128, 2560], BF)
            xa = big.tile([128, 36, 68], BF)
            xb = big.tile([128, 36, 68], BF)
            wb = const.tile([128, 1772], BF)
            fb = const.tile([128, 6], FP)
            ydown = big.tile([128, 34, 66], BF)
            zt = big.tile([68, 36, 256], BF)
            expv = big.tile([100, 32, 64], BF)
            maskv = big.tile([128, 16, 100], BF)
            inv = big.tile([128, 16, 4], FP)
            btX = big.tile([68, 8, 1280], BF)
            bstage = dpool.tile([68, 8, 1280], BF, name="bstage")

            # weight views into the blob
            wdt = wb[:, 0:256].rearrange("p (a b) -> p a b", a=2)      # [128,2,128]
            wet = wb[:, 256:1156].rearrange("p (a b) -> p a b", a=9)   # [128,9,100]
            wot = wb[:, 1156:1668].rearrange("p (a b) -> p a b", a=2)  # [128,2,256]
            saug = wb[0:100, 1668:1772]                                # [100,104]
            bd = fb[:, 0:1]
            be = fb[0:100, 1:2]
            bo = fb[:, 2:4]
            edge = fb[:, 4:6]

            # ---- preamble: input DMAs first, then image zero-init ----
            nc.sync.dma_start(out=xa[:], in_=xs_d[0:128])
            nc.sync.dma_start(out=xb[:], in_=xs_d[128:256])
            nc.scalar.dma_start(out=wb[:], in_=wb_d[:])
            nc.scalar.dma_start(out=fb[:], in_=fb_d[:])
            nc.vector.memset(warm[:], 0.0)
            nc.vector.memset(zero_b[:], 0.0)
            # zero the whole banded image in one broadcast DMA (68 rows x 10240)
            nc.sync.dma_start(
                out=_ap(bstage[:], 0, [[10240, 68], [1, 10240]]),
                in_=_ap(zero_b[:], 0, [[2560, 68], [0, 4], [1, 2560]]),
            )

            # ---- PE warmup spin: keep HAM busy through the input-DMA wait ----
            psW = ExitStack()
            pw = psW.enter_context(tc.tile_pool(name="psW", bufs=1, space="PSUM"))
            wps = pw.tile([128, 128], FP)
            for _ in range(30):
                nc.tensor.matmul(wps[:], warm[:], warm[:], start=True, stop=True)
            psW.close()

            # ---- stage A: y_down [128ch, 34r, 66c] = w_down . x + b_down ----
            psA = ExitStack()
            psApool = psA.enter_context(tc.tile_pool(name="psA", bufs=2, space="PSUM"))
            row_blocks = [(0, 6), (6, 12), (12, 18), (18, 24), (24, 30), (30, 34)]
            for bi, (r0, r1) in enumerate(row_blocks):
                nr = r1 - r0
                pa = psApool.tile([128, 6, 66], FP, tag="A")
                nc.tensor.matmul(
                    pa[:, 0:nr, :], wdt[:, 0, :], xa[:, 1 + r0 : 1 + r1, 1:67],
                    start=True, stop=False,
                )
                nc.tensor.matmul(
                    pa[:, 0:nr, :], wdt[:, 1, :], xb[:, 1 + r0 : 1 + r1, 1:67],
                    start=False, stop=True,
                )
                if r0 == 0:
                    nc.vector.tensor_scalar(
                        ydown[:, 0:1, :], pa[:, 0:1, :], bd, edge[:, 0:1],
                        op0=ALU.add, op1=ALU.mult,
                    )
                    nc.scalar.add(ydown[:, 1:6, :], pa[:, 1:6, :], add=bd)
                elif r1 == 34:
                    nc.vector.tensor_scalar(
                        ydown[:, 33:34, :], pa[:, 3:4, :], bd, edge[:, 1:2],
                        op0=ALU.add, op1=ALU.mult,
                    )
                    nc.scalar.add(ydown[:, 30:33, :], pa[:, 0:3, :], add=bd)
                else:
                    if bi % 2 == 0:
                        nc.vector.tensor_scalar(
                            ydown[:, r0:r1, :], pa[:, 0:nr, :], bd, None, op0=ALU.add
                        )
                    else:
                        nc.scalar.add(ydown[:, r0:r1, :], pa[:, 0:nr, :], add=bd)
            nc.vector.memset(ydown[:, :, 0:1], 0.0)
            nc.vector.memset(ydown[:, :, 65:66], 0.0)
            psA.close()

            psBC = ExitStack()
            psB = psBC.enter_context(tc.tile_pool(name="psB", bufs=2, space="PSUM"))
            psC = psBC.enter_context(tc.tile_pool(name="psC", bufs=2, space="PSUM"))
            psZ = ExitStack()
            psZpool = psZ.enter_context(tc.tile_pool(name="psZ", bufs=2, space="PSUM"))

            # ---- stage B: enc -> exp(enc + b_enc) [100, 32, 64] ----
            for b4 in range(4):
                pb = psB.tile([100, 8, 64], FP, tag="B")
                k = 0
                for di in range(3):
                    for dj in range(3):
                        nc.tensor.matmul(
                            pb[:],
                            wet[:, 3 * di + dj, :],
                            ydown[:, di + 8 * b4 : di + 8 * b4 + 8, dj : dj + 64],
                            start=(k == 0), stop=(k == 8),
                        )
                        k += 1
                nc.scalar.activation(
                    expv[:, 8 * b4 : 8 * b4 + 8, :], pb[:], AF.Exp, bias=be
                )

            # ---- stage C: selector matmul + normalize -> maskv; scatter ----
            expf = expv[:].rearrange("p a b -> p (a b)")
            for kc in range(16):
                pc = psC.tile([128, 104], FP, tag="C")
                nc.tensor.matmul(
                    pc[:], expf[:, 128 * kc : 128 * (kc + 1)], saug,
                    start=True, stop=True,
                )
                nc.vector.reciprocal(inv[:, kc, :], pc[:, 100:104])
                inv_b = _ap(inv[:], kc * 4, [[64, 128], [0, 25], [1, 4]])
                nc.vector.tensor_tensor(
                    maskv[:, kc, :].rearrange("p (k q) -> p k q", q=4),
                    pc[:, 0:100].rearrange("p (k q) -> p k q", q=4),
                    inv_b,
                    op=ALU.mult,
                )
            # scatter/reload pipeline: maskv holds all 16 kc, so the banded
            # image (8 rotating slots) is filled 2 groups ahead of stage D.
            def stage_group(t):
                # scatter h rows 4t..4t+3 (kc = 2t, 2t+1), then reload slots
                for kk in range(2):
                    kc = 2 * t + kk
                    for hh in range(2):
                        dstm = _ap(
                            bstage[:],
                            ((2 * kc + hh) % 8) * 1280,
                            [[10260, 64], [10240, 5], [1, 20]],
                        )
                        seng = nc.gpsimd if hh == 0 else nc.sync
                        seng.dma_start(
                            out=dstm, in_=maskv[64 * hh : 64 * hh + 64, kc, :]
                        )
                s0 = (4 * t) % 8
                nc.sync.dma_start(
                    out=btX[:, s0 : s0 + 4, :],
                    in_=_ap(bstage[:], s0 * 1280, [[10240, 68], [1, 5120]]),
                )

            stage_group(0)
            stage_group(1)

            # ---- stage Z: zT [68col, 36r, 256ch] = (w_out . x)^T ----
            for g in range(9):
                pz = psZpool.tile([68, 4, 256], FP, tag="Z")
                for rr in range(4):
                    r = 4 * g + rr
                    nc.tensor.matmul(
                        pz[:, rr, :], xa[:, r, :], wot[:, 0, :], start=True, stop=False
                    )
                    nc.tensor.matmul(
                        pz[:, rr, :], xb[:, r, :], wot[:, 1, :], start=False, stop=True
                    )
                if g % 2 == 0:
                    nc.vector.tensor_copy(zt[:, 4 * g : 4 * g + 4, :], pz[:])
                else:
                    nc.scalar.copy(zt[:, 4 * g : 4 * g + 4, :], pz[:])
            psZ.close()
            psBC.close()

            # ---- stage D: banded reassembly + b_out ----
            psD = ctx.enter_context(tc.tile_pool(name="psD", bufs=3, space="PSUM"))
            obs = [None, None]
            for h in range(32):
                for chunk in range(2):
                    if h % 8 == 0:
                        obs[chunk] = opool.tile(
                            [128, 8, 2, 128], BF, tag=f"ob{chunk}", name=f"ob{chunk}"
                        )
                    pd = psD.tile([128, 512], FP, tag="D")
                    for i in range(5):
                        rhs = _ap(
                            btX[:],
                            (h % 8) * 1280 + 4 * i,
                            [[10240, 68], [20, 64], [1, 4]],
                        )
                        nc.tensor.matmul(
                            pd[:, 0:256],
                            zt[0:68, h + i, 128 * chunk : 128 * chunk + 128],
                            rhs,
                            start=(i == 0), stop=(i == 4),
                        )
                    ob = obs[chunk]
                    for dh in range(2):
                        src = _ap(pd[:], 2 * dh, [[512, 128], [4, 64], [1, 2]])
                        dst = ob[:, h % 8, dh, :]
                        if dh == 0:
                            nc.vector.tensor_scalar(
                                dst, src, bo[:, chunk : chunk + 1], None, op0=ALU.add
                            )
                        else:
                            nc.scalar.add(dst, src, add=bo[:, chunk : chunk + 1])
                    if h % 8 == 7:
                        nc.scalar.dma_start(
                            out=out_d[
                                128 * chunk : 128 * (chunk + 1),
                                2 * h - 14 : 2 * h + 2,
                                :,
                            ],
                            in_=obs[chunk][:].rearrange("p a d w -> p (a d w)"),
                        )
                if h % 4 == 3 and h // 4 + 2 <= 7:
                    stage_group(h // 4 + 2)

    nc.compile()
    return nc


def _host_prep(x, w_down, b_down, w_enc, b_enc, w_out, b_out):
    import ml_dtypes

    bft = ml_dtypes.bfloat16
    x = np.asarray(x, np.float32)
    xp = np.pad(x, [(0, 0), (0, 0), (2, 2), (2, 2)]).astype(bft)
    wdt = np.ascontiguousarray(np.asarray(w_down, np.float32)[:, :, 0, 0].T)  # [256,128]
    wet = np.ascontiguousarray(
        np.asarray(w_enc, np.float32).transpose(1, 2, 3, 0).reshape(128, 900)
    )
    wot = np.ascontiguousarray(np.asarray(w_out, np.float32)[:, :, 0, 0].T)  # [256,256]
    # saug: permuted identity (e=(i5,j5,p4) -> e'=(j5,i5,p4)) + 4 group-sum cols
    saug = np.zeros((100, 104), np.float32)
    for i in range(5):
        for j in range(5):
            for p in range(4):
                saug[(i * 5 + j) * 4 + p, j * 20 + i * 4 + p] = 1.0
    for e in range(100):
        saug[e, 100 + e % 4] = 1.0
    wb = np.zeros((128, 1772), np.float32)
    wb[:, 0:128] = wdt[0:128]
    wb[:, 128:256] = wdt[128:256]
    wb[:, 256:1156] = wet
    wb[:, 1156:1412] = wot[0:128]
    wb[:, 1412:1668] = wot[128:256]
    wb[0:100, 1668:1772] = saug
    wb = wb.astype(bft)

    bd = np.asarray(b_down, np.float32)
    be = np.asarray(b_enc, np.float32)
    bo = np.asarray(b_out, np.float32)
    in_maps = []
    for c in range(NCORES):
        n, hh = c // 2, c % 2
        xs = np.ascontiguousarray(xp[n, :, hh * 32 : hh * 32 + 36, :])
        fb = np.zeros((128, 6), np.float32)
        fb[:, 0] = bd
        fb[0:100, 1] = be
        fb[:, 2] = bo[0:128]
        fb[:, 3] = bo[128:256]
        fb[:, 4] = 0.0 if hh == 0 else 1.0
        fb[:, 5] = 0.0 if hh == 1 else 1.0
        in_maps.append(dict(xs=xs, wb=wb, fb=fb))
    return in_maps


last_exec_time_ns = None


def kernel(x, w_down, b_down, w_enc, b_enc, w_out, b_out):
    global last_exec_time_ns
    nc = _build()
    in_maps = _host_prep(x, w_down, b_down, w_enc, b_enc, w_out, b_out)
    res = run_bass_kernel_spmd(nc, in_maps, list(range(NCORES)))
    last_exec_time_ns = res.exec_time_ns
    out = np.empty((4, 256, 128, 128), np.float32)
    for c in range(NCORES):
        n, hh = c // 2, c % 2
        out[n, :, hh * 64 : (hh + 1) * 64, :] = np.asarray(
            res.results[c]["out"], np.float32
        )
    return out


# revision 17
# speedup vs baseline: 1.4021x; 1.4021x over previous
"""CARAFE content-aware upsampling on 8 Trainium2 NeuronCores.

Strategy (data parallel): 8 cores = 4 batch images x 2 row-halves
(32 low-res rows each, +2-row halo). Per core, fused pipeline in SBUF:
  A) y_down = conv1x1(x, w_down)+b_down       (PE, K=256 in 2 chunks)
  B) enc -> exp(enc+b_enc)                    (PE conv3x3, 9 shifted matmuls)
  C) mask = softmax over 25 taps (4 groups)   (PE selector matmul -> DVE
     reciprocal+normalize), scattered to a banded DRAM image
  Z) zT = (w_out . x) transposed              (PE produces [col, ch])
  D) out = sum_i zt[h+i] @ B_band(h,i) + b_out (PE, K=68 banded matmuls,
     one MM per (h, tap, ch-half), N=256)
The final 1x1 conv (w_out) is folded BEFORE reassembly (z-trick).

v2 changes vs v1: K=68 single-band stage D (320 MMs vs 1280), PE warmup
spin to hold HAM at 2.4GHz, dense A->B->C||Z->D ordering, bf16 output,
consolidated weight-blob loads, 1 scatter DMA per kc, chunky reloads,
8-slot rotating band image zero-initialized once during the preamble.

Layouts:
  xs     [256, 36, 68]   zero-padded shard (rows h0-2..h1+2, cols -2..65)
  zt     [68, 36, 256]   col-on-partition transpose of z = w_out . x
  btX    [68, 8, 1280]   banded masks, 8 rotating h-slots:
                         btX[w+j, h%8, w*20 + i*4 + p] = mask[h,w,i,j,p]
  out    [256, 64, 128]  hi-res shard (bf16)
"""

import sys
import functools
import numpy as np
from contextlib import ExitStack

for _p in ("/opt/trn_rl_repo",):
    if _p not in sys.path:
        sys.path.insert(0, _p)

import concourse.bass as bass
import concourse.bacc as bacc
import concourse.mybir as mybir
import concourse.tile as tile
from concourse.bass_utils import run_bass_kernel_spmd

NCORES = 8
FP = mybir.dt.float32
BF = mybir.dt.bfloat16
AF = mybir.ActivationFunctionType
ALU = mybir.AluOpType


def _ap(base, offset_delta, dims):
    return bass.AP(tensor=base.tensor, offset=base.offset + offset_delta, ap=dims)


@functools.lru_cache(maxsize=1)
def _build():
    nc = bacc.Bacc("TRN2", target_bir_lowering=False, debug=False, num_devices=NCORES)

    xs_d = nc.declare_dram_parameter("xs", [256, 36, 68], BF, isOutput=False)
    wb_d = nc.declare_dram_parameter("wb", [128, 1772], BF, isOutput=False)
    fb_d = nc.declare_dram_parameter("fb", [128, 6], FP, isOutput=False)
    out_d = nc.declare_dram_parameter("out", [256, 64, 128], BF, isOutput=True)

    with tile.TileContext(nc) as tc:
        with ExitStack() as ctx:
            const = ctx.enter_context(tc.tile_pool(name="const", bufs=1))
            big = ctx.enter_context(tc.tile_pool(name="big", bufs=1))
            opool = ctx.enter_context(tc.tile_pool(name="opool", bufs=4))
            dpool = ctx.enter_context(tc.tile_pool(name="dpool", bufs=1, space="DRAM"))

            # ---- tiles ----
            warm = const.tile([128, 128], BF)
            zero_b = const.tile([128, 2560], BF)
            xa = big.tile([128, 36, 68], BF)
            xb = big.tile([128, 36, 68], BF)
            wb = const.tile([128, 1772], BF)
            fb = const.tile([128, 6], FP)
            ydown = big.tile([128, 34, 66], BF)
            zt = big.tile([68, 36, 256], BF)
            expv = big.tile([100, 32, 64], BF)
            maskv = big.tile([128, 16, 100], BF)
            inv = big.tile([128, 16, 4], FP)
            btX = big.tile([68, 16, 1280], BF)
            bstage = dpool.tile([68, 16, 1280], BF, name="bstage")

            # weight views into the blob
            wdt = wb[:, 0:256].rearrange("p (a b) -> p a b", a=2)      # [128,2,128]
            wet = wb[:, 256:1156].rearrange("p (a b) -> p a b", a=9)   # [128,9,100]
            wot = wb[:, 1156:1668].rearrange("p (a b) -> p a b", a=2)  # [128,2,256]
            saug = wb[0:100, 1668:1772]                                # [100,104]
            bd = fb[:, 0:1]
            be = fb[0:100, 1:2]
            bo = fb[:, 2:4]
            edge = fb[:, 4:6]

            # ---- preamble: input DMAs first, then image zero-init ----
            nc.sync.dma_start(out=xa[:], in_=xs_d[0:128])
            nc.sync.dma_start(out=xb[:], in_=xs_d[128:256])
            nc.scalar.dma_start(out=wb[:], in_=wb_d[:])
            nc.scalar.dma_start(out=fb[:], in_=fb_d[:])
            nc.vector.memset(warm[:], 0.0)
            nc.vector.memset(zero_b[:], 0.0)
            # zero only the two banded-image block regions (w<32 -> rows 0:36
            # cols 0:640; w>=32 -> rows 32:68 cols 640:1280), one DMA each
            nc.sync.dma_start(
                out=_ap(bstage[:], 0, [[20480, 36], [1280, 16], [1, 640]]),
                in_=_ap(zero_b[:], 0, [[2560, 36], [0, 16], [1, 640]]),
            )
            nc.sync.dma_start(
                out=_ap(bstage[:], 32 * 20480 + 640, [[20480, 36], [1280, 16], [1, 640]]),
                in_=_ap(zero_b[:], 0, [[2560, 36], [0, 16], [1, 640]]),
            )
            # btX regions outside the reload blocks must be zero (read by MMs)
            nc.vector.memset(btX[32:64, :, 0:640], 0.0)
            nc.vector.memset(btX[64:68, :, 0:640], 0.0)
            nc.scalar.mul(btX[0:32, :, 640:1280], btX[0:32, :, 640:1280], 0.0)

            # ---- PE warmup spin: keep HAM busy through the input-DMA wait ----
            psW = ExitStack()
            pw = psW.enter_context(tc.tile_pool(name="psW", bufs=1, space="PSUM"))
            wps = pw.tile([128, 128], FP)
            for _ in range(30):
                nc.tensor.matmul(wps[:], warm[:], warm[:], start=True, stop=True)
            psW.close()

            # ---- stage A: y_down [128ch, 34r, 66c] = w_down . x + b_down ----
            psA = ExitStack()
            psApool = psA.enter_context(tc.tile_pool(name="psA", bufs=2, space="PSUM"))
            row_blocks = [(0, 6), (6, 12), (12, 18), (18, 24), (24, 30), (30, 34)]
            for bi, (r0, r1) in enumerate(row_blocks):
                nr = r1 - r0
                pa = psApool.tile([128, 6, 66], FP, tag="A")
                nc.tensor.matmul(
                    pa[:, 0:nr, :], wdt[:, 0, :], xa[:, 1 + r0 : 1 + r1, 1:67],
                    start=True, stop=False,
                )
                nc.tensor.matmul(
                    pa[:, 0:nr, :], wdt[:, 1, :], xb[:, 1 + r0 : 1 + r1, 1:67],
                    start=False, stop=True,
                )
                if r0 == 0:
                    nc.vector.tensor_scalar(
                        ydown[:, 0:1, :], pa[:, 0:1, :], bd, edge[:, 0:1],
                        op0=ALU.add, op1=ALU.mult,
                    )
                    nc.scalar.add(ydown[:, 1:6, :], pa[:, 1:6, :], add=bd)
                elif r1 == 34:
                    nc.vector.tensor_scalar(
                        ydown[:, 33:34, :], pa[:, 3:4, :], bd, edge[:, 1:2],
                        op0=ALU.add, op1=ALU.mult,
                    )
                    nc.scalar.add(ydown[:, 30:33, :], pa[:, 0:3, :], add=bd)
                else:
                    if bi % 2 == 0:
                        nc.vector.tensor_scalar(
                            ydown[:, r0:r1, :], pa[:, 0:nr, :], bd, None, op0=ALU.add
                        )
                    else:
                        nc.scalar.add(ydown[:, r0:r1, :], pa[:, 0:nr, :], add=bd)
            nc.vector.memset(ydown[:, :, 0:1], 0.0)
            nc.vector.memset(ydown[:, :, 65:66], 0.0)
            psA.close()

            psBC = ExitStack()
            psB = psBC.enter_context(tc.tile_pool(name="psB", bufs=2, space="PSUM"))
            psC = psBC.enter_context(tc.tile_pool(name="psC", bufs=2, space="PSUM"))
            psZ = ExitStack()
            psZpool = psZ.enter_context(tc.tile_pool(name="psZ", bufs=2, space="PSUM"))

            # ---- stage B: enc -> exp(enc + b_enc) [100, 32, 64] ----
            for b4 in range(4):
                pb = psB.tile([100, 8, 64], FP, tag="B")
                k = 0
                for di in range(3):
                    for dj in range(3):
                        nc.tensor.matmul(
                            pb[:],
                            wet[:, 3 * di + dj, :],
                            ydown[:, di + 8 * b4 : di + 8 * b4 + 8, dj : dj + 64],
                            start=(k == 0), stop=(k == 8),
                        )
                        k += 1
                nc.scalar.activation(
                    expv[:, 8 * b4 : 8 * b4 + 8, :], pb[:], AF.Exp, bias=be
                )

            # scatter/reload pipeline: maskv holds all 16 kc; the banded
            # image + btX rotate 16 h-slots, staged 4 groups ahead of stage D.
            def stage_group(t):
                s0 = (4 * t) % 16
                # scatter: one DMA per (kc, hh) half-row, 3-dim diagonal AP
                for kk in range(2):
                    for hh in range(2):
                        dstm = _ap(
                            bstage[:],
                            (s0 + 2 * kk + hh) * 1280,
                            [[20500, 64], [20480, 5], [1, 20]],
                        )
                        seng = nc.sync if hh == 1 else nc.scalar
                        seng.dma_start(
                            out=dstm,
                            in_=maskv[64 * hh : 64 * hh + 64, 2 * t + kk, :],
                        )
                nc.sync.dma_start(
                    out=btX[0:36, s0 : s0 + 4, 0:640],
                    in_=_ap(bstage[:], s0 * 1280, [[20480, 36], [1280, 4], [1, 640]]),
                )
                nc.scalar.dma_start(
                    out=btX[32:68, s0 : s0 + 4, 640:1280],
                    in_=_ap(
                        bstage[:],
                        32 * 20480 + s0 * 1280 + 640,
                        [[20480, 36], [1280, 4], [1, 640]],
                    ),
                )

            # ---- stage C: selector matmul + normalize -> maskv; scatter ----
            expf = expv[:].rearrange("p a b -> p (a b)")
            for kc in range(16):
                pc = psC.tile([128, 104], FP, tag="C")
                nc.tensor.matmul(
                    pc[:], expf[:, 128 * kc : 128 * (kc + 1)], saug,
                    start=True, stop=True,
                )
                nc.vector.reciprocal(inv[:, kc, :], pc[:, 100:104])
                inv_b = _ap(inv[:], kc * 4, [[64, 128], [0, 25], [1, 4]])
                nc.vector.tensor_tensor(
                    maskv[:, kc, :].rearrange("p (k q) -> p k q", q=4),
                    pc[:, 0:100].rearrange("p (k q) -> p k q", q=4),
                    inv_b,
                    op=ALU.mult,
                )
                if kc % 2 == 1 and kc <= 7:
                    stage_group(kc // 2)
            # ---- stage Z: zT [68col, 36r, 256ch] = (w_out . x)^T ----
            for g in range(9):
                pz = psZpool.tile([68, 4, 256], FP, tag="Z")
                for rr in range(4):
                    r = 4 * g + rr
                    nc.tensor.matmul(
                        pz[:, rr, :], xa[:, r, :], wot[:, 0, :], start=True, stop=False
                    )
                    nc.tensor.matmul(
                        pz[:, rr, :], xb[:, r, :], wot[:, 1, :], start=False, stop=True
                    )
                if g % 2 == 0:
                    nc.vector.tensor_copy(zt[:, 4 * g : 4 * g + 4, :], pz[:])
                else:
                    nc.scalar.copy(zt[:, 4 * g : 4 * g + 4, :], pz[:])
            psZ.close()
            psBC.close()

            # ---- stage D: banded reassembly + b_out ----
            psD = ctx.enter_context(tc.tile_pool(name="psD", bufs=3, space="PSUM"))
            obs = [None, None]
            for h in range(32):
                for chunk in range(2):
                    if h % 8 == 0:
                        obs[chunk] = opool.tile(
                            [128, 8, 2, 128], BF, tag=f"ob{chunk}", name=f"ob{chunk}"
                        )
                    pd = psD.tile([128, 512], FP, tag="D")
                    for i in range(5):
                        rhs = _ap(
                            btX[:],
                            (h % 16) * 1280 + 4 * i,
                            [[20480, 68], [20, 64], [1, 4]],
                        )
                        nc.tensor.matmul(
                            pd[:, 0:256],
                            zt[0:68, h + i, 128 * chunk : 128 * chunk + 128],
                            rhs,
                            start=(i == 0), stop=(i == 4),
                        )
                    ob = obs[chunk]
                    srcv = _ap(pd[:], 0, [[512, 128], [2, 2], [4, 64], [1, 2]])
                    dst = ob[:, h % 8, :, :].rearrange("p d w -> p (d w)")
                    if (h + chunk) % 2 == 0:
                        nc.vector.tensor_scalar(
                            dst, srcv, bo[:, chunk : chunk + 1], None, op0=ALU.add
                        )
                    else:
                        nc.scalar.add(dst, srcv, add=bo[:, chunk : chunk + 1])
                    if h % 8 == 7:
                        nc.scalar.dma_start(
                            out=out_d[
                                128 * chunk : 128 * (chunk + 1),
                                2 * h - 14 : 2 * h + 2,
                                :,
                            ],
                            in_=obs[chunk][:].rearrange("p a d w -> p (a d w)"),
                        )
                if h % 4 == 3 and h // 4 + 4 <= 7:
                    stage_group(h // 4 + 4)

    nc.compile()
    return nc


def _host_prep(x, w_down, b_down, w_enc, b_enc, w_out, b_out):
    import ml_dtypes

    bft = ml_dtypes.bfloat16
    x = np.asarray(x, np.float32)
    xp = np.pad(x, [(0, 0), (0, 0), (2, 2), (2, 2)]).astype(bft)
    wdt = np.ascontiguousarray(np.asarray(w_down, np.float32)[:, :, 0, 0].T)  # [256,128]
    wet = np.ascontiguousarray(
        np.asarray(w_enc, np.float32).transpose(1, 2, 3, 0).reshape(128, 900)
    )
    wot = np.ascontiguousarray(np.asarray(w_out, np.float32)[:, :, 0, 0].T)  # [256,256]
    # saug: permuted identity (e=(i5,j5,p4) -> e'=(j5,i5,p4)) + 4 group-sum cols
    saug = np.zeros((100, 104), np.float32)
    for i in range(5):
        for j in range(5):
            for p in range(4):
                saug[(i * 5 + j) * 4 + p, j * 20 + i * 4 + p] = 1.0
    for e in range(100):
        saug[e, 100 + e % 4] = 1.0
    wb = np.zeros((128, 1772), np.float32)
    wb[:, 0:128] = wdt[0:128]
    wb[:, 128:256] = wdt[128:256]
    wb[:, 256:1156] = wet
    wb[:, 1156:1412] = wot[0:128]
    wb[:, 1412:1668] = wot[128:256]
    wb[0:100, 1668:1772] = saug
    wb = wb.astype(bft)

    bd = np.asarray(b_down, np.float32)
    be = np.asarray(b_enc, np.float32)
    bo = np.asarray(b_out, np.float32)
    in_maps = []
    for c in range(NCORES):
        n, hh = c // 2, c % 2
        xs = np.ascontiguousarray(xp[n, :, hh * 32 : hh * 32 + 36, :])
        fb = np.zeros((128, 6), np.float32)
        fb[:, 0] = bd
        fb[0:100, 1] = be
        fb[:, 2] = bo[0:128]
        fb[:, 3] = bo[128:256]
        fb[:, 4] = 0.0 if hh == 0 else 1.0
        fb[:, 5] = 0.0 if hh == 1 else 1.0
        in_maps.append(dict(xs=xs, wb=wb, fb=fb))
    return in_maps


last_exec_time_ns = None


def kernel(x, w_down, b_down, w_enc, b_enc, w_out, b_out):
    global last_exec_time_ns
    nc = _build()
    in_maps = _host_prep(x, w_down, b_down, w_enc, b_enc, w_out, b_out)
    res = run_bass_kernel_spmd(nc, in_maps, list(range(NCORES)))
    last_exec_time_ns = res.exec_time_ns
    out = np.empty((4, 256, 128, 128), np.float32)
    for c in range(NCORES):
        n, hh = c // 2, c % 2
        out[n, :, hh * 64 : (hh + 1) * 64, :] = np.asarray(
            res.results[c]["out"], np.float32
        )
    return out
